# revision 1
# baseline (speedup 1.0000x reference)
"""Forward-fill scan kernel for Trainium2 (8 NeuronCores).

Problem: baseline[i] = baseline[i-1] if wet[i] else att[i]; baseline[0] = att[0],
independently per row of a (256, 131072) batch.

Strategy:
  - Shard the 256 rows across 8 cores (32 rows/core).
  - Split each row into 4 segments of 32768 so each core fills all 128 SBUF
    partitions (partition p = 4*local_row + segment).
  - The scan is z[t] = wet[t]*z[t-1] + dry[t]*att[t], an affine recurrence the
    DVE's tensor_tensor_scan instruction computes natively along the free dim.
  - Segment-boundary carry-in values (att at the last reset index before the
    segment, 1024 scalars total) are gathered on the host and passed as a tiny
    [128,1] input per core, which decouples all partitions.
  - Device streams chunks of F columns: DMA in -> cast wet->f32 (scalar engine)
    -> b = att*dry (gpsimd) -> tensor_tensor_scan (vector engine, carry chained
    through the previous chunk's last column) -> DMA out.
"""

import numpy as np

B, S = 256, 131072
N_CORES = 8
ROWS_PER_CORE = B // N_CORES  # 32
NSEG = 4                      # segments per row -> 32*4 = 128 partitions
SEG = S // NSEG               # 32768
P = 128
F = 4096                      # columns per streamed chunk
BUFS = 2


def _build_program(n_part=P, seg=SEG, f=F, bufs=BUFS, n_cores=N_CORES,
                   u8_scan=False, stt_b=False, b_engine="gpsimd"):
    """Emit the SPMD Bass/Tile program (same program runs on every core)."""
    import concourse.bacc as bacc
    import concourse.mybir as mybir
    from concourse import tile

    nc = bacc.Bacc("TRN2", target_bir_lowering=False, debug=False,
                   num_devices=n_cores)
    att = nc.dram_tensor("att", [n_part, seg], mybir.dt.float32,
                         kind="ExternalInput").ap()
    wet = nc.dram_tensor("wet", [n_part, seg], mybir.dt.uint8,
                         kind="ExternalInput").ap()
    init = nc.dram_tensor("init", [n_part, 1], mybir.dt.float32,
                          kind="ExternalInput").ap()
    out = nc.dram_tensor("out", [n_part, seg], mybir.dt.float32,
                         kind="ExternalOutput").ap()

    nchunk = seg // f
    assert seg % f == 0
    fdt = mybir.dt.float32
    Copy = mybir.ActivationFunctionType.Copy
    Op = mybir.AluOpType

    with tile.TileContext(nc) as tc:
        with tc.tile_pool(name="cpool", bufs=1) as cp, \
             tc.tile_pool(name="work", bufs=bufs) as wp:
            c_t = cp.tile([n_part, 1], fdt)
            nc.sync.dma_start(c_t[:, :], init[:, :])
            z_prev = None
            for k in range(nchunk):
                sl = slice(k * f, (k + 1) * f)
                att_t = wp.tile([n_part, f], fdt, tag="att")
                nc.sync.dma_start(att_t[:, :], att[:, sl])
                wet_t = wp.tile([n_part, f], mybir.dt.uint8, tag="wet")
                nc.sync.dma_start(wet_t[:, :], wet[:, sl])

                b_eng = nc.gpsimd if b_engine == "gpsimd" else nc.vector
                b_t = wp.tile([n_part, f], fdt, tag="b")
                if stt_b:
                    # b = (wet == 0) * att, fused
                    b_eng.scalar_tensor_tensor(b_t[:, :], wet_t[:, :], 0.0,
                                               att_t[:, :], Op.is_equal, Op.mult)
                else:
                    dryf_t = wp.tile([n_part, f], fdt, tag="dryf")
                    nc.scalar.activation(dryf_t[:, :], wet_t[:, :], Copy,
                                         bias=1.0, scale=-1.0)
                    b_eng.tensor_tensor(b_t[:, :], att_t[:, :], dryf_t[:, :],
                                        op=Op.mult)

                if u8_scan:
                    scan_w = wet_t
                else:
                    wetf_t = wp.tile([n_part, f], fdt, tag="wetf")
                    nc.scalar.activation(wetf_t[:, :], wet_t[:, :], Copy)
                    scan_w = wetf_t

                z_t = wp.tile([n_part, f], fdt, tag="z")
                initial = c_t[:, 0:1] if k == 0 else z_prev[:, f - 1:f]
                nc.vector.tensor_tensor_scan(z_t[:, :], scan_w[:, :], b_t[:, :],
                                             initial, Op.mult, Op.add)
                nc.sync.dma_start(out[:, sl], z_t[:, :])
                z_prev = z_t
    nc.compile()
    return nc


def _host_carries(att, wet_dry, nseg):
    """att value at the last reset index before each segment start (bit-exact
    gather; no float arithmetic). init[:, 0] is att[:, 0] (index 0 is always a
    reset)."""
    b, s = att.shape
    seg = s // nseg
    reset = np.logical_not(wet_dry)
    reset[:, 0] = True
    idx = np.arange(s, dtype=np.int64)
    cand = np.where(reset, idx, 0)
    segmax = cand.reshape(b, nseg, seg).max(axis=2)
    run = np.maximum.accumulate(segmax, axis=1)
    init = np.empty((b, nseg), np.float32)
    init[:, 0] = att[:, 0]
    if nseg > 1:
        init[:, 1:] = att[np.arange(b)[:, None], run[:, :-1]]
    return init


def _prep_in_maps(att, wet_dry, n_cores=N_CORES, nseg=NSEG):
    b, s = att.shape
    rows = b // n_cores
    seg = s // nseg
    n_part = rows * nseg
    init = _host_carries(att, wet_dry, nseg)
    wet_u8 = np.ascontiguousarray(wet_dry).view(np.uint8)
    in_maps = []
    for c in range(n_cores):
        r0, r1 = c * rows, (c + 1) * rows
        in_maps.append({
            "att": np.ascontiguousarray(att[r0:r1]).reshape(n_part, seg),
            "wet": wet_u8[r0:r1].reshape(n_part, seg),
            "init": init[r0:r1].reshape(n_part, 1),
        })
    return in_maps


def kernel(input_attenuation, input_wet_dry):
    att = np.ascontiguousarray(np.asarray(input_attenuation, dtype=np.float32))
    wet_dry = np.ascontiguousarray(np.asarray(input_wet_dry, dtype=bool))
    assert att.shape == (B, S) and wet_dry.shape == (B, S)

    in_maps = _prep_in_maps(att, wet_dry)
    nc = _build_program()

    from concourse.bass_utils import run_bass_kernel_spmd
    res = run_bass_kernel_spmd(nc, in_maps, list(range(N_CORES)))

    out = np.empty((B, S), np.float32)
    for c in range(N_CORES):
        r0 = c * ROWS_PER_CORE
        out[r0:r0 + ROWS_PER_CORE] = res.results[c]["out"].reshape(
            ROWS_PER_CORE, S)
    return out
